# revision 1
# baseline (speedup 1.0000x reference)
"""Causal multi-headed attention (conv-q/k, linear-v, causal softmax, out-proj)
as a Bass/Tile SPMD kernel for 8 Trainium2 NeuronCores.

Sharding: core c -> (batch b = c // 2, head-group g = c % 2).  Each core
processes one batch and 4 of the 8 heads (256 of 512 channels).  The final
linear is computed as per-core partial products over the core's 256 channels;
the host sums the two head-group partials per batch and adds all biases that
commute with the structure (lin1 bias folds through softmax-sums-to-one into
the output bias; lin2 bias added once on host).

Layout: everything on-chip is kept "feature-on-partition" (transposed), so
scores are computed as scores_T[s_k, s_q] = k_hat^T q_hat with dk on the
contraction dim, exp runs on ScalarE straight out of PSUM, the softmax
denominator rides as an appended ones-column on V (M=65 matmuls), and
normalization happens with a partition_broadcast + vector multiply.

All matmul operands are bf16 (host-cast, which also halves HBM traffic);
accumulation is fp32 in PSUM; the returned output is fp32.
"""

import math

import numpy as np
import ml_dtypes

import concourse.bass as bass
import concourse.mybir as mybir
from concourse import bacc, tile
from concourse.bass_utils import run_bass_kernel_spmd

F32 = mybir.dt.float32
BF16 = mybir.dt.bfloat16
BF = ml_dtypes.bfloat16

S = 2048          # sequence length
B = 4             # batch
D = 512           # model dim
NH_TOT = 8        # total heads
NH = 4            # heads per core
DK = 64           # head dim
KS = 3            # conv kernel size
CI_CH = 4         # 512 input channels / 128
CO_CH = 2         # 256 output channels / 128
QC = 4            # s_q chunks of 512
SCALE = 1.0 / math.sqrt(DK)
N_CORES = 8

_CACHE = {}


def _build_program():
    nc = bacc.Bacc("TRN2", target_bir_lowering=False, debug=False)

    xq_d = nc.dram_tensor("xq", [128, CI_CH, S + 2], BF16, kind="ExternalInput")
    xk_d = nc.dram_tensor("xk", [128, CI_CH, S + 2], BF16, kind="ExternalInput")
    xv_d = nc.dram_tensor("xv", [128, CI_CH, S], BF16, kind="ExternalInput")
    wq_d = nc.dram_tensor("wq", [128, KS, CI_CH, 256], BF16, kind="ExternalInput")
    wk_d = nc.dram_tensor("wk", [128, KS, CI_CH, 256], BF16, kind="ExternalInput")
    wv_d = nc.dram_tensor("wv", [128, CI_CH, 256], BF16, kind="ExternalInput")
    wo_d = nc.dram_tensor("wo", [64, NH, 4, 128], BF16, kind="ExternalInput")
    bq_d = nc.dram_tensor("bq", [128, CO_CH], F32, kind="ExternalInput")
    bk_d = nc.dram_tensor("bk", [128, CO_CH], F32, kind="ExternalInput")
    tri_d = nc.dram_tensor("tri", [128, 128], BF16, kind="ExternalInput")
    out_d = nc.dram_tensor("out", [128, 4, S], F32, kind="ExternalOutput")

    Exp = mybir.ActivationFunctionType.Exp

    with tile.TileContext(nc) as tc:
        with (
            tc.tile_pool(name="consts", bufs=1) as consts,
            tc.tile_pool(name="xin", bufs=1) as xin,
            tc.tile_pool(name="acts", bufs=1) as acts,
            tc.tile_pool(name="ppool", bufs=3) as ppool,
            tc.tile_pool(name="xnpool", bufs=6) as xnpool,
            tc.tile_pool(name="rpool", bufs=3) as rpool,
            tc.tile_pool(name="rbpool", bufs=3) as rbpool,
            tc.tile_pool(name="outpool", bufs=2) as outpool,
        ):
            wq_s = consts.tile([128, KS, CI_CH, 256], BF16)
            wk_s = consts.tile([128, KS, CI_CH, 256], BF16)
            wv_s = consts.tile([128, CI_CH, 256], BF16)
            wo_s = consts.tile([64, NH, 4, 128], BF16)
            bq_s = consts.tile([128, CO_CH], F32)
            bk_s = consts.tile([128, CO_CH], F32)
            tri_s = consts.tile([128, 128], BF16)
            nc.sync.dma_start(out=wq_s[:], in_=wq_d[:])
            nc.sync.dma_start(out=wk_s[:], in_=wk_d[:])
            nc.sync.dma_start(out=wv_s[:], in_=wv_d[:])
            nc.sync.dma_start(out=wo_s[:], in_=wo_d[:])
            nc.sync.dma_start(out=bq_s[:], in_=bq_d[:])
            nc.sync.dma_start(out=bk_s[:], in_=bk_d[:])
            nc.sync.dma_start(out=tri_s[:], in_=tri_d[:])

            xq_s = xin.tile([128, CI_CH, S + 2], BF16)
            xk_s = xin.tile([128, CI_CH, S + 2], BF16)
            xv_s = xin.tile([128, CI_CH, S], BF16)
            nc.sync.dma_start(out=xq_s[:], in_=xq_d[:])
            nc.sync.dma_start(out=xk_s[:], in_=xk_d[:])
            nc.sync.dma_start(out=xv_s[:], in_=xv_d[:])

            qT_s = acts.tile([128, CO_CH, S], BF16)
            kT_s = acts.tile([128, CO_CH, S], BF16)
            # v in natural [s, c] layout; dim1 = kc*NH + h, last col = ones
            v_s = acts.tile([128, 16 * NH, DK + 1], BF16)

            # ---- causal convs for q-hat (pre-scaled by 1/sqrt(dk)) and k-hat
            with tc.tile_pool(name="cvps", bufs=2, space="PSUM") as cvps:
                for x_s, w_s, b_s, y_s in (
                    (xq_s, wq_s, bq_s, qT_s),
                    (xk_s, wk_s, bk_s, kT_s),
                ):
                    for cc in range(CO_CH):
                        for sc in range(QC):
                            ps = cvps.tile([128, 512], F32, tag="cv")
                            first = True
                            for t in range(KS):
                                for c in range(CI_CH):
                                    nc.tensor.matmul(
                                        ps[:],
                                        w_s[:, t, c, cc * 128:(cc + 1) * 128],
                                        x_s[:, c, sc * 512 + t: sc * 512 + t + 512],
                                        start=first,
                                        stop=(t == KS - 1 and c == CI_CH - 1),
                                    )
                                    first = False
                            nc.vector.tensor_scalar_add(
                                y_s[:, cc, sc * 512:(sc + 1) * 512],
                                ps[:],
                                b_s[:, cc:cc + 1],
                            )

                # ---- v = lin1 @ x_v in natural [s, c] layout + ones column
                for sc in range(16):
                    ps = cvps.tile([128, 512], F32, tag="cv")
                    for c in range(CI_CH):
                        nc.tensor.matmul(
                            ps[:, 0:256],
                            xv_s[:, c, sc * 128:(sc + 1) * 128],
                            wv_s[:, c, :],
                            start=(c == 0),
                            stop=(c == CI_CH - 1),
                        )
                    nc.vector.tensor_copy(
                        v_s[:, sc * NH:(sc + 1) * NH, 0:DK],
                        ps[:, 0:256].rearrange("p (h d) -> p h d", h=NH),
                    )
                nc.vector.memset(v_s[:, :, DK:DK + 1], 1.0)

            # ---- attention + output projection, one s_q chunk at a time
            with (
                tc.tile_pool(name="scps", bufs=2, space="PSUM") as scps,
                tc.tile_pool(name="accps", bufs=2, space="PSUM") as accps,
            ):
                for qc in range(QC):
                    xn_tiles = []
                    for h in range(NH):
                        nk = 4 * (qc + 1)
                        prow = (h % 2) * 64
                        cc = h // 2
                        xaug = accps.tile([128, 512], F32, tag="acc")
                        ngroups = (nk + 2) // 3
                        for kg in range(ngroups):
                            n = min(3, nk - 3 * kg)
                            scp = scps.tile([128, 3, 512], F32, tag="sc")
                            pt = ppool.tile([128, 3, 512], BF16, tag="p")
                            for j in range(n):
                                kc = 3 * kg + j
                                nc.tensor.matmul(
                                    scp[:, j, :],
                                    kT_s[prow:prow + 64, cc,
                                         kc * 128:(kc + 1) * 128],
                                    qT_s[prow:prow + 64, cc,
                                         qc * 512:(qc + 1) * 512],
                                    start=True,
                                    stop=True,
                                )
                            nc.scalar.activation(pt[:, 0:n, :], scp[:, 0:n, :], Exp)
                            for j in range(n):
                                kc = 3 * kg + j
                                jj = kc - 4 * qc
                                if jj >= 0:  # diagonal block: causal mask
                                    if jj >= 1:
                                        nc.gpsimd.memset(pt[:, j, 0:128 * jj], 0.0)
                                    nc.vector.tensor_mul(
                                        pt[:, j, 128 * jj:128 * jj + 128],
                                        pt[:, j, 128 * jj:128 * jj + 128],
                                        tri_s[:],
                                    )
                            for j in range(n):
                                kc = 3 * kg + j
                                nc.tensor.matmul(
                                    xaug[0:65, :],
                                    v_s[:, kc * NH + h, :],
                                    pt[:, j, :],
                                    start=(kc == 0),
                                    stop=(kc == nk - 1),
                                )
                        rrow = rpool.tile([1, 512], F32, tag="r")
                        nc.vector.reciprocal(rrow[:], xaug[64:65, :])
                        rb = rbpool.tile([64, 512], F32, tag="rb")
                        nc.gpsimd.partition_broadcast(rb[:], rrow[:])
                        xn = xnpool.tile([64, 512], BF16, tag="xn")
                        nc.vector.tensor_mul(xn[:], xaug[0:64, :], rb[:])
                        xn_tiles.append(xn)

                    osb = outpool.tile([128, 4, 512], F32, tag="o")
                    for dc in range(4):
                        lp = accps.tile([128, 512], F32, tag="acc")
                        for h in range(NH):
                            nc.tensor.matmul(
                                lp[:],
                                wo_s[:, h, dc, :],
                                xn_tiles[h][:],
                                start=(h == 0),
                                stop=(h == NH - 1),
                            )
                        nc.vector.tensor_copy(osb[:, dc, :], lp[:])
                    nc.sync.dma_start(
                        out=out_d[:, :, qc * 512:(qc + 1) * 512], in_=osb[:]
                    )

    nc.finalize()
    return nc


def _prep_core_inputs(query, key, value, conv1_w, conv1_b, conv2_w, conv2_b,
                      lin1_w):
    """Host-side shard + layout transform.  Returns in_maps for the 8 cores."""
    in_maps = []
    for c in range(N_CORES):
        b, g = c // 2, c % 2
        co0 = 256 * g

        def feat_part(x):  # [S, D] f32 -> [128, CI_CH, S] bf16 (ci%128, ci//128, s)
            t = np.ascontiguousarray(x.T).reshape(CI_CH, 128, -1)
            return np.ascontiguousarray(t.transpose(1, 0, 2)).astype(BF)

        xq = np.zeros((128, CI_CH, S + 2), dtype=BF)
        xk = np.zeros((128, CI_CH, S + 2), dtype=BF)
        xq[:, :, 2:] = feat_part(query[:, b, :])
        xk[:, :, 2:] = feat_part(key[:, b, :])
        xv = feat_part(value[:, b, :])

        # wq[p, t, c, o] = conv1_w[co0+o, c*128+p, t] * SCALE
        wcut = conv1_w[co0:co0 + 256].astype(np.float32) * SCALE  # [256, 512, 3]
        wq = np.ascontiguousarray(
            wcut.reshape(256, CI_CH, 128, KS).transpose(2, 3, 1, 0)).astype(BF)
        wcut = conv2_w[co0:co0 + 256].astype(np.float32)
        wk = np.ascontiguousarray(
            wcut.reshape(256, CI_CH, 128, KS).transpose(2, 3, 1, 0)).astype(BF)
        # wv[p, c, o] = lin1_w[co0+o, c*128+p]
        wv = np.ascontiguousarray(
            lin1_w[co0:co0 + 256].reshape(256, CI_CH, 128).transpose(2, 1, 0)
        ).astype(BF)
        bq = np.ascontiguousarray(
            (conv1_b[co0:co0 + 256].astype(np.float32) * SCALE).reshape(2, 128).T)
        bk = np.ascontiguousarray(
            conv2_b[co0:co0 + 256].astype(np.float32).reshape(2, 128).T)
        tri = np.triu(np.ones((128, 128), dtype=np.float32)).astype(BF)

        in_maps.append({
            "xq": xq, "xk": xk, "xv": xv,
            "wq": wq, "wk": wk, "wv": wv,
            "wo": _CACHE["wo_maps"][g],
            "bq": bq, "bk": bk, "tri": tri,
        })
    return in_maps


def kernel(query, key, value, attn_mask,
           conv1_w, conv1_b, conv2_w, conv2_b,
           lin1_w, lin1_b, lin2_w, lin2_b):
    if "nc" not in _CACHE:
        _CACHE["nc"] = _build_program()

    # wo[p, h, dc, d'] = lin2_w[dc*128+d', 256g + h*64 + p]   (per head-group)
    wo_maps = []
    for g in range(2):
        wcut = lin2_w[:, 256 * g:256 * g + 256].astype(np.float32)  # [512(d), 256(c)]
        wo = np.ascontiguousarray(
            wcut.reshape(4, 128, NH, 64).transpose(3, 2, 0, 1)).astype(BF)
        wo_maps.append(wo)
    _CACHE["wo_maps"] = wo_maps

    in_maps = _prep_core_inputs(query, key, value, conv1_w, conv1_b,
                                conv2_w, conv2_b, lin1_w)

    res = run_bass_kernel_spmd(_CACHE["nc"], in_maps, list(range(N_CORES)))
    _CACHE["last_results"] = res

    # total bias: lin2_b + lin2_w @ lin1_b (softmax rows sum to one, so the
    # lin1 bias passes straight through attention into the output projection)
    total_bias = (lin2_b.astype(np.float64)
                  + lin2_w.astype(np.float64) @ lin1_b.astype(np.float64))

    out = np.empty((S, B, D), dtype=np.float32)
    for b in range(B):
        acc = None
        for g in range(2):
            arr = res.results[2 * b + g]["out"]  # [128, 4, S]
            part = arr.transpose(1, 0, 2).reshape(D, S)  # [D, S]
            acc = part if acc is None else acc + part
        out[:, b, :] = (acc.T + total_bias[None, :]).astype(np.float32)
    return out


# revision 9
# speedup vs baseline: 8663.5758x; 8663.5758x over previous
"""Causal multi-headed attention (conv-q/k, linear-v, causal softmax, out-proj)
as a Bass/Tile SPMD kernel for 8 Trainium2 NeuronCores.

Sharding: core c -> (batch b = c // 2, head-group g = c % 2).  Each core
processes one batch and 4 of the 8 heads (256 of 512 channels).  The final
linear is computed as per-core partial products over the core's 256 channels;
the host sums the two head-group partials per batch and adds all biases that
commute with the structure (lin1 bias folds through softmax-sums-to-one into
the output bias; lin2 bias added once on host).

Layout: everything on-chip is kept "feature-on-partition" (transposed), so
scores are computed as scores_T[s_k, s_q] = k_hat^T q_hat with dk on the
contraction dim, exp runs on ScalarE straight out of PSUM, the softmax
denominator rides as an appended ones-column on V (M=65 matmuls), and
normalization happens with a partition_broadcast + vector multiply.

All matmul operands are bf16 (host-cast, which also halves HBM traffic);
accumulation is fp32 in PSUM; the returned output is fp32.
"""

import math

import numpy as np
import ml_dtypes

import jax
import jax.numpy as jnp
from jax.experimental.shard_map import shard_map
from jax.sharding import Mesh, NamedSharding, PartitionSpec

import concourse.bass as bass
import concourse.mybir as mybir
from concourse import bacc, bass2jax, tile

F32 = mybir.dt.float32
BF16 = mybir.dt.bfloat16
BF = ml_dtypes.bfloat16

S = 2048          # sequence length
B = 4             # batch
D = 512           # model dim
NH_TOT = 8        # total heads
NH = 4            # heads per core
DK = 64           # head dim
KS = 3            # conv kernel size
CI_CH = 4         # 512 input channels / 128
CO_CH = 2         # 256 output channels / 128
QC = 4            # s_q chunks of 512
SCALE = 1.0 / math.sqrt(DK)
N_CORES = 8

_CACHE = {}


def _build_program():
    nc = bacc.Bacc("TRN2", target_bir_lowering=False, debug=False)

    xq_d = nc.dram_tensor("xq", [128, CI_CH, S + 2], BF16, kind="ExternalInput")
    xk_d = nc.dram_tensor("xk", [128, CI_CH, S + 2], BF16, kind="ExternalInput")
    xv_d = nc.dram_tensor("xv", [128, CI_CH, S], BF16, kind="ExternalInput")
    wq_d = nc.dram_tensor("wq", [128, KS, CI_CH, 256], BF16, kind="ExternalInput")
    wk_d = nc.dram_tensor("wk", [128, KS, CI_CH, 256], BF16, kind="ExternalInput")
    wv_d = nc.dram_tensor("wv", [128, CI_CH, 256], BF16, kind="ExternalInput")
    wo_d = nc.dram_tensor("wo", [64, NH, 4, 128], BF16, kind="ExternalInput")
    bq_d = nc.dram_tensor("bq", [128, CO_CH], F32, kind="ExternalInput")
    bk_d = nc.dram_tensor("bk", [128, CO_CH], F32, kind="ExternalInput")
    tri_d = nc.dram_tensor("tri", [128, 128], BF16, kind="ExternalInput")
    out_d = nc.dram_tensor("out", [128, 4, S], F32, kind="ExternalOutput")

    Exp = mybir.ActivationFunctionType.Exp

    with tile.TileContext(nc) as tc:
        with (
            tc.tile_pool(name="consts", bufs=1) as consts,
            tc.tile_pool(name="xin", bufs=1) as xin,
            tc.tile_pool(name="acts", bufs=1) as acts,
            tc.tile_pool(name="ppool", bufs=3) as ppool,
            tc.tile_pool(name="xnpool", bufs=6) as xnpool,
            tc.tile_pool(name="rpool", bufs=3) as rpool,
            tc.tile_pool(name="rbpool", bufs=3) as rbpool,
            tc.tile_pool(name="outpool", bufs=2) as outpool,
        ):
            wq_s = consts.tile([128, KS, CI_CH, 256], BF16)
            wk_s = consts.tile([128, KS, CI_CH, 256], BF16)
            wv_s = consts.tile([128, CI_CH, 256], BF16)
            wo_s = consts.tile([64, NH, 4, 128], BF16)
            bq_s = consts.tile([128, CO_CH], F32)
            bk_s = consts.tile([128, CO_CH], F32)
            tri_s = consts.tile([128, 128], BF16)
            nc.sync.dma_start(out=wq_s[:], in_=wq_d[:])
            nc.sync.dma_start(out=wk_s[:], in_=wk_d[:])
            nc.sync.dma_start(out=wv_s[:], in_=wv_d[:])
            nc.sync.dma_start(out=wo_s[:], in_=wo_d[:])
            nc.sync.dma_start(out=bq_s[:], in_=bq_d[:])
            nc.sync.dma_start(out=bk_s[:], in_=bk_d[:])
            nc.sync.dma_start(out=tri_s[:], in_=tri_d[:])

            xq_s = xin.tile([128, CI_CH, S + 2], BF16)
            xk_s = xin.tile([128, CI_CH, S + 2], BF16)
            xv_s = xin.tile([128, CI_CH, S], BF16)
            nc.sync.dma_start(out=xq_s[:], in_=xq_d[:])
            nc.sync.dma_start(out=xk_s[:], in_=xk_d[:])
            nc.sync.dma_start(out=xv_s[:], in_=xv_d[:])

            qT_s = acts.tile([128, CO_CH, S], BF16)
            kT_s = acts.tile([128, CO_CH, S], BF16)
            # v in natural [s, c] layout; dim1 = kc*NH + h, last col = ones
            v_s = acts.tile([128, 16 * NH, DK + 1], BF16)

            # ---- causal convs for q-hat (pre-scaled by 1/sqrt(dk)) and k-hat
            with tc.tile_pool(name="cvps", bufs=2, space="PSUM") as cvps:
                for x_s, w_s, b_s, y_s in (
                    (xq_s, wq_s, bq_s, qT_s),
                    (xk_s, wk_s, bk_s, kT_s),
                ):
                    for cc in range(CO_CH):
                        for sc in range(QC):
                            ps = cvps.tile([128, 512], F32, tag="cv")
                            first = True
                            for t in range(KS):
                                for c in range(CI_CH):
                                    nc.tensor.matmul(
                                        ps[:],
                                        w_s[:, t, c, cc * 128:(cc + 1) * 128],
                                        x_s[:, c, sc * 512 + t: sc * 512 + t + 512],
                                        start=first,
                                        stop=(t == KS - 1 and c == CI_CH - 1),
                                    )
                                    first = False
                            nc.vector.tensor_scalar_add(
                                y_s[:, cc, sc * 512:(sc + 1) * 512],
                                ps[:],
                                b_s[:, cc:cc + 1],
                            )

                # ---- v = lin1 @ x_v in natural [s, c] layout + ones column
                for sc in range(16):
                    ps = cvps.tile([128, 512], F32, tag="cv")
                    for c in range(CI_CH):
                        nc.tensor.matmul(
                            ps[:, 0:256],
                            xv_s[:, c, sc * 128:(sc + 1) * 128],
                            wv_s[:, c, :],
                            start=(c == 0),
                            stop=(c == CI_CH - 1),
                        )
                    nc.vector.tensor_copy(
                        v_s[:, sc * NH:(sc + 1) * NH, 0:DK],
                        ps[:, 0:256].rearrange("p (h d) -> p h d", h=NH),
                    )
                nc.vector.memset(v_s[:, :, DK:DK + 1], 1.0)

            # ---- attention + output projection, one s_q chunk at a time
            with (
                tc.tile_pool(name="scps", bufs=2, space="PSUM") as scps,
                tc.tile_pool(name="accps", bufs=2, space="PSUM") as accps,
            ):
                for qc in range(QC):
                    xn_tiles = []
                    for h in range(NH):
                        nk = 4 * (qc + 1)
                        prow = (h % 2) * 64
                        cc = h // 2
                        xaug = accps.tile([128, 512], F32, tag="acc")
                        ngroups = (nk + 2) // 3
                        for kg in range(ngroups):
                            n = min(3, nk - 3 * kg)
                            scp = scps.tile([128, 3, 512], F32, tag="sc")
                            pt = ppool.tile([128, 3, 512], BF16, tag="p")
                            for j in range(n):
                                kc = 3 * kg + j
                                nc.tensor.matmul(
                                    scp[:, j, :],
                                    kT_s[prow:prow + 64, cc,
                                         kc * 128:(kc + 1) * 128],
                                    qT_s[prow:prow + 64, cc,
                                         qc * 512:(qc + 1) * 512],
                                    start=True,
                                    stop=True,
                                )
                            nc.scalar.activation(pt[:, 0:n, :], scp[:, 0:n, :], Exp)
                            for j in range(n):
                                kc = 3 * kg + j
                                jj = kc - 4 * qc
                                if jj >= 0:  # diagonal block: causal mask
                                    if jj >= 1:
                                        nc.gpsimd.memset(pt[:, j, 0:128 * jj], 0.0)
                                    nc.vector.tensor_mul(
                                        pt[:, j, 128 * jj:128 * jj + 128],
                                        pt[:, j, 128 * jj:128 * jj + 128],
                                        tri_s[:],
                                    )
                            for j in range(n):
                                kc = 3 * kg + j
                                nc.tensor.matmul(
                                    xaug[0:65, :],
                                    v_s[:, kc * NH + h, :],
                                    pt[:, j, :],
                                    start=(kc == 0),
                                    stop=(kc == nk - 1),
                                )
                        rrow = rpool.tile([1, 512], F32, tag="r")
                        nc.vector.reciprocal(rrow[:], xaug[64:65, :])
                        rb = rbpool.tile([64, 512], F32, tag="rb")
                        nc.gpsimd.partition_broadcast(rb[:], rrow[:])
                        xn = xnpool.tile([64, 512], BF16, tag="xn")
                        nc.vector.tensor_mul(xn[:], xaug[0:64, :], rb[:])
                        xn_tiles.append(xn)

                    osb = outpool.tile([128, 4, 512], F32, tag="o")
                    for dc in range(4):
                        lp = accps.tile([128, 512], F32, tag="acc")
                        for h in range(NH):
                            nc.tensor.matmul(
                                lp[:],
                                wo_s[:, h, dc, :],
                                xn_tiles[h][:],
                                start=(h == 0),
                                stop=(h == NH - 1),
                            )
                        nc.vector.tensor_copy(osb[:, dc, :], lp[:])
                    nc.sync.dma_start(
                        out=out_d[:, :, qc * 512:(qc + 1) * 512], in_=osb[:]
                    )

    nc.finalize()
    return nc


class _Runner:
    """Cached jit executor for a Bass program over the 8 axon cores.

    Mirrors bass2jax.run_bass_via_pjrt but keeps the jitted function and
    device placement reusable across calls (no donation: every output
    element is written by the kernel, so uninit result buffers are fine).
    """

    def __init__(self, nc):
        bass2jax.install_neuronx_cc_hook()
        self.nc = nc
        part_name = (nc.partition_id_tensor.name
                     if nc.partition_id_tensor else None)
        in_names, out_names, out_avals, zero_outs = [], [], [], []
        for alloc in nc.m.functions[0].allocations:
            if not isinstance(alloc, mybir.MemoryLocationSet):
                continue
            name = alloc.memorylocations[0].name
            if alloc.kind == "ExternalInput":
                if name != part_name:
                    in_names.append(name)
            elif alloc.kind == "ExternalOutput":
                out_names.append(name)
                shape = tuple(alloc.tensor_shape)
                dtype = mybir.dt.np(alloc.dtype)
                out_avals.append(jax.core.ShapedArray(shape, dtype))
                zero_outs.append(np.zeros(shape, dtype))
        self.n_params = len(in_names)
        self.in_names = in_names + out_names
        if part_name is not None:
            self.in_names.append(part_name)
        self.out_names = out_names
        self.out_avals = out_avals
        self.zero_outs = zero_outs

        devices = jax.devices()[:N_CORES]
        self.mesh = Mesh(np.asarray(devices), ("core",))
        self.sharding = NamedSharding(self.mesh, PartitionSpec("core"))
        n_all = self.n_params + len(out_names)
        in_specs = (PartitionSpec("core"),) * n_all
        out_specs = (PartitionSpec("core"),) * len(out_names)

        in_names_t = tuple(self.in_names)
        out_names_t = tuple(out_names)
        out_avals_t = tuple(out_avals)

        def _body(*args):
            operands = list(args)
            if part_name is not None:
                operands.append(bass2jax.partition_id_tensor())
            outs = bass2jax._bass_exec_p.bind(
                *operands,
                out_avals=out_avals_t,
                in_names=in_names_t,
                out_names=out_names_t,
                lowering_input_output_aliases=(),
                sim_require_finite=True,
                sim_require_nnan=True,
                nc=nc,
            )
            return tuple(outs)

        self.fn = jax.jit(
            shard_map(_body, mesh=self.mesh, in_specs=in_specs,
                      out_specs=out_specs, check_rep=False),
            keep_unused=True,
        )
        self._zero_dev = None

    def put(self, in_maps):
        """Concatenate per-core inputs along axis 0 and move to devices."""
        concat = [
            np.concatenate([np.asarray(m[name]) for m in in_maps], axis=0)
            for name in self.in_names[: self.n_params]
        ]
        dev = [jax.device_put(a, self.sharding) for a in concat]
        if self._zero_dev is None:
            self._zero_dev = [
                jax.device_put(
                    np.zeros((N_CORES * z.shape[0], *z.shape[1:]), z.dtype),
                    self.sharding,
                )
                for z in self.zero_outs
            ]
        return dev + self._zero_dev

    def run_dev(self, dev_args):
        outs = self.fn(*dev_args)
        jax.block_until_ready(outs)
        return outs

    def run(self, in_maps):
        outs = self.run_dev(self.put(in_maps))
        return [
            {
                name: np.asarray(outs[i]).reshape(
                    N_CORES, *self.out_avals[i].shape)[c]
                for i, name in enumerate(self.out_names)
            }
            for c in range(N_CORES)
        ]


def _build_noop():
    nc = bacc.Bacc("TRN2", target_bir_lowering=False, debug=False)
    a = nc.dram_tensor("a", [128, 8], F32, kind="ExternalInput")
    o = nc.dram_tensor("o", [128, 8], F32, kind="ExternalOutput")
    with tile.TileContext(nc) as tc:
        with tc.tile_pool(name="sb", bufs=1) as sb:
            t = sb.tile([128, 8], F32)
            nc.sync.dma_start(out=t[:], in_=a[:])
            nc.sync.dma_start(out=o[:], in_=t[:])
    nc.finalize()
    return nc


def get_runner():
    if "runner" not in _CACHE:
        _CACHE["runner"] = _Runner(_build_program())
    return _CACHE["runner"]


def get_noop_runner():
    if "noop" not in _CACHE:
        _CACHE["noop"] = _Runner(_build_noop())
    return _CACHE["noop"]


def _prep_core_inputs(query, key, value, conv1_w, conv1_b, conv2_w, conv2_b,
                      lin1_w):
    """Host-side shard + layout transform.  Returns in_maps for the 8 cores."""
    in_maps = []
    for c in range(N_CORES):
        b, g = c // 2, c % 2
        co0 = 256 * g

        def feat_part(x):  # [S, D] f32 -> [128, CI_CH, S] bf16 (ci%128, ci//128, s)
            t = np.ascontiguousarray(x.T).reshape(CI_CH, 128, -1)
            return np.ascontiguousarray(t.transpose(1, 0, 2)).astype(BF)

        xq = np.zeros((128, CI_CH, S + 2), dtype=BF)
        xk = np.zeros((128, CI_CH, S + 2), dtype=BF)
        xq[:, :, 2:] = feat_part(query[:, b, :])
        xk[:, :, 2:] = feat_part(key[:, b, :])
        xv = feat_part(value[:, b, :])

        # wq[p, t, c, o] = conv1_w[co0+o, c*128+p, t] * SCALE
        wcut = conv1_w[co0:co0 + 256].astype(np.float32) * SCALE  # [256, 512, 3]
        wq = np.ascontiguousarray(
            wcut.reshape(256, CI_CH, 128, KS).transpose(2, 3, 1, 0)).astype(BF)
        wcut = conv2_w[co0:co0 + 256].astype(np.float32)
        wk = np.ascontiguousarray(
            wcut.reshape(256, CI_CH, 128, KS).transpose(2, 3, 1, 0)).astype(BF)
        # wv[p, c, o] = lin1_w[co0+o, c*128+p]
        wv = np.ascontiguousarray(
            lin1_w[co0:co0 + 256].reshape(256, CI_CH, 128).transpose(2, 1, 0)
        ).astype(BF)
        bq = np.ascontiguousarray(
            (conv1_b[co0:co0 + 256].astype(np.float32) * SCALE).reshape(2, 128).T)
        bk = np.ascontiguousarray(
            conv2_b[co0:co0 + 256].astype(np.float32).reshape(2, 128).T)
        tri = np.triu(np.ones((128, 128), dtype=np.float32)).astype(BF)

        in_maps.append({
            "xq": xq, "xk": xk, "xv": xv,
            "wq": wq, "wk": wk, "wv": wv,
            "wo": _CACHE["wo_maps"][g],
            "bq": bq, "bk": bk, "tri": tri,
        })
    return in_maps


def kernel(query, key, value, attn_mask,
           conv1_w, conv1_b, conv2_w, conv2_b,
           lin1_w, lin1_b, lin2_w, lin2_b):
    runner = get_runner()

    # wo[p, h, dc, d'] = lin2_w[dc*128+d', 256g + h*64 + p]   (per head-group)
    wo_maps = []
    for g in range(2):
        wcut = lin2_w[:, 256 * g:256 * g + 256].astype(np.float32)  # [512(d), 256(c)]
        wo = np.ascontiguousarray(
            wcut.reshape(4, 128, NH, 64).transpose(3, 2, 0, 1)).astype(BF)
        wo_maps.append(wo)
    _CACHE["wo_maps"] = wo_maps

    in_maps = _prep_core_inputs(query, key, value, conv1_w, conv1_b,
                                conv2_w, conv2_b, lin1_w)

    results = runner.run(in_maps)
    _CACHE["last_in_maps"] = in_maps

    # total bias: lin2_b + lin2_w @ lin1_b (softmax rows sum to one, so the
    # lin1 bias passes straight through attention into the output projection)
    total_bias = (lin2_b.astype(np.float64)
                  + lin2_w.astype(np.float64) @ lin1_b.astype(np.float64))

    out = np.empty((S, B, D), dtype=np.float32)
    for b in range(B):
        acc = None
        for g in range(2):
            arr = results[2 * b + g]["out"]  # [128, 4, S]
            part = arr.transpose(1, 0, 2).reshape(D, S)  # [D, S]
            acc = part if acc is None else acc + part
        out[:, b, :] = (acc.T + total_bias[None, :]).astype(np.float32)
    return out
